# revision 4
# baseline (speedup 1.0000x reference)
"""DeepseekV3 MLA decode attention kernel for 8 Trainium2 NeuronCores.

Sharding: 4 head-groups (32 heads each) x 2 batch-groups (8 batches each).
Each core computes the full attention output for its (head-group, batch-group)
tile. Weights are sharded by head, KV cache by batch. All matmul operands are
bf16 (fp32 PSUM accumulation); softmax runs in fp32.

Per-core pipeline:
  1. q = q_dn @ wq^T                (bf16 matmul, fp32 psum)
  2. RoPE on q_pe and new-token k_pe (DVE, fp32)
  3. PE transposes of q_nope/q_pe per head -> [d, b] layout
  4. absorption: q_lat^T[c,b] = w_ukv[h]^T-slices @ q_nope^T
  5. per batch: scores = q_lat.ckv^T + q_pe.kpe^T  -> exp (ACT, accum sums)
     -> transpose p -> o = p^T.T @ ckv -> scale by 1/sum
  6. out[b,d] = o^T-slices @ w_uv^T per head
"""

import sys

for _p in ("/opt/trn_rl_repo", "/root/.axon_site/_ro/trn_rl_repo"):
    if _p not in sys.path:
        sys.path.append(_p)

import numpy as np
import ml_dtypes

import concourse.bass as bass
import concourse.bacc as bacc
import concourse.tile as tile
from concourse import mybir
from concourse.bass_utils import run_bass_kernel_spmd
from concourse.masks import make_identity

BF16 = mybir.dt.bfloat16
F32 = mybir.dt.float32
NPBF = ml_dtypes.bfloat16

NUM_HEADS = 128
QK_NOPE = 128
QK_ROPE = 64
V_HEAD = 128
QD = 192  # q head dim (nope + rope)
C = 512   # kv lora rank
L = 1536  # q lora rank
SCALE = 1.0 / float(np.sqrt(192.0))

HG = 4  # head groups
BGQ = 2  # batch groups
N_CORES = 8

_BUILD_CACHE = {}


def _build(n_cached, B, H):
    """Build the per-core Bass program. Identical on all cores (pure SPMD)."""
    NT_T = n_cached // 128   # full 128-row n tiles (16)
    NCH = n_cached // 512    # 512-wide score chunks (4)
    HD = H * QD              # 6144
    LT = L // 128            # 12
    NJ = HD // 512           # 12
    assert n_cached % 512 == 0

    nc = bacc.Bacc("TRN2", target_bir_lowering=False, debug=False)

    q_dnT = nc.dram_tensor("q_dnT", [L, B], BF16, kind="ExternalInput")
    wqT = nc.dram_tensor("wqT", [L, HD], BF16, kind="ExternalInput")
    w_ukv = nc.dram_tensor("w_ukv", [H, QK_NOPE, C], BF16, kind="ExternalInput")
    w_uvT = nc.dram_tensor("w_uvT", [H, C, V_HEAD], BF16, kind="ExternalInput")
    ckv = nc.dram_tensor("ckv", [B, n_cached, C], BF16, kind="ExternalInput")
    ckvT = nc.dram_tensor("ckvT", [B, C, n_cached], BF16, kind="ExternalInput")
    kpeT = nc.dram_tensor("kpeT", [B, QK_ROPE, n_cached], BF16, kind="ExternalInput")
    ckv_new = nc.dram_tensor("ckv_new", [1, B, C], BF16, kind="ExternalInput")
    ckv_newT = nc.dram_tensor("ckv_newT", [C, B], BF16, kind="ExternalInput")
    kpe_new = nc.dram_tensor("kpe_new", [B, QK_ROPE], F32, kind="ExternalInput")
    cos_rep = nc.dram_tensor("cos_rep", [B, H * 32], F32, kind="ExternalInput")
    sin_rep = nc.dram_tensor("sin_rep", [B, H * 32], F32, kind="ExternalInput")
    out = nc.dram_tensor("out", [B, H, V_HEAD], F32, kind="ExternalOutput")

    with tile.TileContext(nc) as tc:
        # Outer (whole-kernel-lifetime) pools. The big cache-streaming pools
        # are opened first so their SBUF addresses never overlap the phase-A
        # scratch pools -> their DMAs can start at t=0.
        with (
            tc.tile_pool(name="ckvT_p", bufs=4) as ckvT_p,
            tc.tile_pool(name="ckv_p", bufs=4) as ckv_p,
            tc.tile_pool(name="kpeT_p", bufs=2) as kpeT_p,
            tc.tile_pool(name="consts", bufs=1) as consts,
            tc.tile_pool(name="persist", bufs=1) as persist,
        ):
            ident = consts.tile([128, 128], BF16)
            make_identity(nc, ident)
            cos_sb = consts.tile([B, H * 32], F32)
            nc.sync.dma_start(out=cos_sb, in_=cos_rep[:, :])
            sin_sb = consts.tile([B, H * 32], F32)
            nc.sync.dma_start(out=sin_sb, in_=sin_rep[:, :])
            kpnew_sb = consts.tile([B, QK_ROPE], F32)
            nc.sync.dma_start(out=kpnew_sb, in_=kpe_new[:, :])
            qdn_sb = consts.tile([128, LT, B], BF16)
            nc.sync.dma_start(
                out=qdn_sb, in_=q_dnT[:, :].rearrange("(t p) b -> p t b", p=128)
            )
            ckvnewT_sb = consts.tile([128, 4, B], BF16)
            nc.sync.dma_start(
                out=ckvnewT_sb, in_=ckv_newT[:, :].rearrange("(ct p) b -> p ct b", p=128)
            )
            ckvnew_sb2 = consts.tile([1, B, C], BF16)
            nc.sync.dma_start(out=ckvnew_sb2, in_=ckv_new[:, :, :])

            # persistent intermediates
            qlatT = persist.tile([128, 4, H, B], BF16)
            qpeT = persist.tile([QK_ROPE, H, B], BF16)
            knewT = persist.tile([QK_ROPE, B], BF16)

            # ---------------- Phase A: q projection, rope, transposes -------
            with (
                tc.tile_pool(name="s1a", bufs=1) as s1a,
                tc.tile_pool(name="wq_p", bufs=4) as wq_p,
                tc.tile_pool(name="wukv_p", bufs=4) as wukv_p,
                tc.tile_pool(name="ps_q", bufs=2, space="PSUM") as ps_q,
                tc.tile_pool(name="ps_t", bufs=2, space="PSUM") as ps_t,
            ):
                q_sb = s1a.tile([B, HD], BF16)
                for j in range(NJ):
                    psq = ps_q.tile([B, 512], F32, tag="psq")
                    for t in range(LT):
                        wqt = wq_p.tile([128, 512], BF16, tag="wq")
                        nc.sync.dma_start(
                            out=wqt, in_=wqT[t * 128:(t + 1) * 128, j * 512:(j + 1) * 512]
                        )
                        nc.tensor.matmul(
                            psq, lhsT=qdn_sb[:, t, :], rhs=wqt,
                            start=(t == 0), stop=(t == LT - 1),
                        )
                    nc.vector.tensor_copy(q_sb[:, j * 512:(j + 1) * 512], psq)

                qv = q_sb.rearrange("b (h d) -> b h d", d=QD)
                # rope on q_pe: interleaved pairs -> half-split rotated layout
                xpairs = qv[:, :, QK_NOPE:].rearrange("b h (i two) -> b h i two", two=2)
                xe = xpairs[:, :, :, 0]
                xo = xpairs[:, :, :, 1]
                cos3 = cos_sb.rearrange("b (h i) -> b h i", i=32)
                sin3 = sin_sb.rearrange("b (h i) -> b h i", i=32)
                qpe_bf = s1a.tile([B, H, QK_ROPE], BF16)
                tmp = s1a.tile([B, 4, H, 32], F32)
                nc.vector.tensor_mul(tmp[:, 0], xe, cos3)
                nc.vector.tensor_mul(tmp[:, 1], xo, sin3)
                nc.vector.tensor_sub(qpe_bf[:, :, 0:32], tmp[:, 0], tmp[:, 1])
                nc.vector.tensor_mul(tmp[:, 2], xo, cos3)
                nc.vector.tensor_mul(tmp[:, 3], xe, sin3)
                nc.vector.tensor_add(qpe_bf[:, :, 32:64], tmp[:, 2], tmp[:, 3])

                # rope on the new-token k_pe
                kpairs = kpnew_sb.rearrange("b (i two) -> b i two", two=2)
                kxe = kpairs[:, :, 0]
                kxo = kpairs[:, :, 1]
                kr_bf = s1a.tile([B, QK_ROPE], BF16)
                ktmp = s1a.tile([B, 4, 32], F32)
                nc.vector.tensor_mul(ktmp[:, 0], kxe, cos3[:, 0, :])
                nc.vector.tensor_mul(ktmp[:, 1], kxo, sin3[:, 0, :])
                nc.vector.tensor_sub(kr_bf[:, 0:32], ktmp[:, 0], ktmp[:, 1])
                nc.vector.tensor_mul(ktmp[:, 2], kxo, cos3[:, 0, :])
                nc.vector.tensor_mul(ktmp[:, 3], kxe, sin3[:, 0, :])
                nc.vector.tensor_add(kr_bf[:, 32:64], ktmp[:, 2], ktmp[:, 3])

                # transposes: [B, d] -> [d, B] per head
                qnT = s1a.tile([128, H, B], BF16)
                for h in range(H):
                    pt1 = ps_t.tile([128, B], BF16, tag="tr")
                    nc.tensor.transpose(pt1, qv[:, h, 0:QK_NOPE], ident[:B, :B])
                    nc.vector.tensor_copy(qnT[:, h, :], pt1)
                    pt2 = ps_t.tile([128, B], BF16, tag="tr")
                    nc.tensor.transpose(pt2[:QK_ROPE], qpe_bf[:, h, :], ident[:B, :B])
                    nc.vector.tensor_copy(qpeT[:, h, :], pt2[:QK_ROPE])
                ptk = ps_t.tile([128, B], BF16, tag="tr")
                nc.tensor.transpose(ptk[:QK_ROPE], kr_bf, ident[:B, :B])
                nc.vector.tensor_copy(knewT, ptk[:QK_ROPE])

                # absorption: q_latT[c, b] per head
                for h in range(H):
                    wut = wukv_p.tile([128, C], BF16, tag="wukv")
                    nc.sync.dma_start(
                        out=wut, in_=w_ukv[h].rearrange("d c -> d c")
                    )
                    for ct in range(4):
                        pa = ps_t.tile([128, B], F32, tag="abs")
                        nc.tensor.matmul(
                            pa, lhsT=wut[:, ct * 128:(ct + 1) * 128], rhs=qnT[:, h, :],
                            start=True, stop=True,
                        )
                        nc.vector.tensor_copy(qlatT[:, ct, h, :], pa)

            # ---------------- Phase B: attention per batch ------------------
            with (
                tc.tile_pool(name="p_p", bufs=2) as p_p,
                tc.tile_pool(name="pT_p", bufs=2) as pT_p,
                tc.tile_pool(name="o_p", bufs=2) as o_p,
                tc.tile_pool(name="oT_p", bufs=1) as oT_p,
                tc.tile_pool(name="sum_p", bufs=2) as sum_p,
                tc.tile_pool(name="wuv_p", bufs=4) as wuv_p,
                tc.tile_pool(name="outs_p", bufs=1) as outs_p,
                tc.tile_pool(name="ps_s", bufs=2, space="PSUM") as ps_s,
                tc.tile_pool(name="ps_pt", bufs=2, space="PSUM") as ps_pt,
                tc.tile_pool(name="ps_o", bufs=1, space="PSUM") as ps_o,
                tc.tile_pool(name="ps_r", bufs=2, space="PSUM") as ps_r,
            ):
                oT = oT_p.tile([128, 4, H, B], BF16)
                for b in range(B):
                    ckvT_t = ckvT_p.tile([128, 4, n_cached], BF16, tag="ckvT")
                    nc.sync.dma_start(
                        out=ckvT_t,
                        in_=ckvT[b].rearrange("(ct p) n -> p ct n", p=128),
                    )
                    kpeT_t = kpeT_p.tile([QK_ROPE, n_cached], BF16, tag="kpeT")
                    nc.sync.dma_start(out=kpeT_t, in_=kpeT[b])

                    p_bf = p_p.tile([32, n_cached], BF16, tag="p")
                    p_tail = p_p.tile([32, 1], BF16, tag="ptail")
                    sums = sum_p.tile([32, 8], F32, tag="sums")
                    # scores + exp, 512-wide chunks
                    for nch in range(NCH):
                        pss = ps_s.tile([32, 512], F32, tag="s")
                        for ct in range(4):
                            nc.tensor.matmul(
                                pss,
                                lhsT=qlatT[:, ct, :, b],
                                rhs=ckvT_t[:, ct, nch * 512:(nch + 1) * 512],
                                start=(ct == 0), stop=False,
                            )
                        nc.tensor.matmul(
                            pss, lhsT=qpeT[:, :, b],
                            rhs=kpeT_t[:, nch * 512:(nch + 1) * 512],
                            start=False, stop=True,
                        )
                        nc.scalar.activation(
                            p_bf[:, nch * 512:(nch + 1) * 512], pss,
                            mybir.ActivationFunctionType.Exp,
                            scale=SCALE, accum_out=sums[:, nch:nch + 1],
                        )
                    # new-token column
                    pst = ps_s.tile([32, 512], F32, tag="s")
                    for ct in range(4):
                        nc.tensor.matmul(
                            pst[:, 0:1], lhsT=qlatT[:, ct, :, b],
                            rhs=ckvnewT_sb[:, ct, b:b + 1],
                            start=(ct == 0), stop=False,
                        )
                    nc.tensor.matmul(
                        pst[:, 0:1], lhsT=qpeT[:, :, b], rhs=knewT[:, b:b + 1],
                        start=False, stop=True,
                    )
                    nc.scalar.activation(
                        p_tail, pst[:, 0:1],
                        mybir.ActivationFunctionType.Exp,
                        scale=SCALE, accum_out=sums[:, NCH:NCH + 1],
                    )
                    # 1 / sum
                    ssum = sum_p.tile([32, 1], F32, tag="ssum")
                    nc.vector.reduce_sum(ssum, sums[:, 0:NCH + 1], axis=mybir.AxisListType.X)
                    rcp = sum_p.tile([32, 1], F32, tag="rcp")
                    nc.vector.reciprocal(rcp, ssum)

                    # transpose p -> pT tiles
                    pT = pT_p.tile([128, NT_T, 32], BF16, tag="pT")
                    for nt in range(NT_T):
                        ptp = ps_pt.tile([128, 32], BF16, tag="pt")
                        nc.tensor.transpose(
                            ptp, p_bf[:, nt * 128:(nt + 1) * 128], ident[:32, :32]
                        )
                        nc.vector.tensor_copy(pT[:, nt, :], ptp)
                    ptt = ps_pt.tile([128, 32], BF16, tag="pt")
                    nc.tensor.transpose(ptt[0:1], p_tail, ident[:32, :32])
                    pT_tail = pT_p.tile([1, 32], BF16, tag="pTt")
                    nc.vector.tensor_copy(pT_tail, ptt[0:1])

                    # o = p @ ckv   (accumulate over n tiles)
                    pso = ps_o.tile([32, C], F32, tag="o")
                    for g in range(NT_T // 4):
                        ckv_t = ckv_p.tile([128, 4, C], BF16, tag="ckv")
                        nc.sync.dma_start(
                            out=ckv_t,
                            in_=ckv[b, g * 512:(g + 1) * 512, :].rearrange(
                                "(nt p) c -> p nt c", p=128
                            ),
                        )
                        for k in range(4):
                            nt = g * 4 + k
                            nc.tensor.matmul(
                                pso, lhsT=pT[:, nt, :], rhs=ckv_t[:, k, :],
                                start=(nt == 0), stop=False,
                            )
                    nc.tensor.matmul(
                        pso, lhsT=pT_tail, rhs=ckvnew_sb2[:, b, :],
                        start=False, stop=True,
                    )
                    # o / sum -> bf16
                    o_bf = o_p.tile([32, C], BF16, tag="obf")
                    nc.vector.tensor_scalar_mul(o_bf, pso, rcp)
                    # transpose o -> oT[:, ct, :, b]
                    for ct in range(4):
                        pto = ps_pt.tile([128, 32], BF16, tag="pt")
                        nc.tensor.transpose(
                            pto, o_bf[:, ct * 128:(ct + 1) * 128], ident[:32, :32]
                        )
                        nc.vector.tensor_copy(oT[:, ct, :, b], pto)

                # ---------------- output projection ------------------------
                out_sb = outs_p.tile([B, H, V_HEAD], F32)
                for h in range(H):
                    wvt = wuv_p.tile([128, 4, V_HEAD], BF16, tag="wuv")
                    nc.sync.dma_start(
                        out=wvt, in_=w_uvT[h].rearrange("(ct p) d -> p ct d", p=128)
                    )
                    psr = ps_r.tile([B, V_HEAD], F32, tag="r")
                    for ct in range(4):
                        nc.tensor.matmul(
                            psr, lhsT=oT[:, ct, h, :], rhs=wvt[:, ct, :],
                            start=(ct == 0), stop=(ct == 3),
                        )
                    nc.vector.tensor_copy(out_sb[:, h, :], psr)
                nc.sync.dma_start(out=out[:, :, :], in_=out_sb)

    nc.compile()
    return nc


def _get_build(n_cached, B, H):
    key = (n_cached, B, H)
    if key not in _BUILD_CACHE:
        _BUILD_CACHE[key] = _build(n_cached, B, H)
    return _BUILD_CACHE[key]


def prepare_in_maps(**inputs):
    """Host-side sharding / layout prep. Returns (in_maps, meta)."""
    q = np.asarray(inputs["q_normed_dn"], dtype=np.float32)      # [16,1,1536]
    ckv_new = np.asarray(inputs["compressed_kv"], dtype=np.float32)  # [16,1,512]
    k_pe = np.asarray(inputs["k_pe"], dtype=np.float32)          # [16,1,1,64]
    pos = np.asarray(inputs["position_ids"]).astype(np.int64)    # [16,1]
    start_pos = int(inputs["start_pos"])
    ckv_cache = np.asarray(inputs["ckv_cache"], dtype=np.float32)
    kpe_cache = np.asarray(inputs["k_pe_cache"], dtype=np.float32)
    sin_c = np.asarray(inputs["sin_cache"], dtype=np.float32)
    cos_c = np.asarray(inputs["cos_cache"], dtype=np.float32)
    wkv_b = np.asarray(inputs["wkv_b"], dtype=np.float32)        # [128,256,512]
    wq_b = np.asarray(inputs["wq_b"], dtype=np.float32)          # [24576,1536]

    bsz = q.shape[0]
    B = bsz // BGQ
    H = NUM_HEADS // HG
    n_cached = start_pos

    cos_g = cos_c[pos[:, 0]][:, :32]                             # [16,32]
    sin_g = sin_c[pos[:, 0]][:, :32]
    cos_rep = np.tile(cos_g, (1, H)).astype(np.float32)          # [16,H*32]
    sin_rep = np.tile(sin_g, (1, H)).astype(np.float32)

    wq_r = wq_b.reshape(NUM_HEADS, QD, L)

    # per head-group weights
    wq_shards, wukv_shards, wuv_shards = [], [], []
    for hg in range(HG):
        hs = slice(hg * H, (hg + 1) * H)
        wq_shards.append(
            np.ascontiguousarray(wq_r[hs].reshape(H * QD, L).T).astype(NPBF)
        )
        wukv_shards.append(np.ascontiguousarray(wkv_b[hs, :QK_NOPE, :]).astype(NPBF))
        wuv_shards.append(
            np.ascontiguousarray(wkv_b[hs, QK_NOPE:, :].transpose(0, 2, 1)).astype(NPBF)
        )

    # per batch-group caches
    ckv_shards, ckvT_shards, kpeT_shards = [], [], []
    qT_shards, ckvnew_shards, ckvnewT_shards, kpnew_shards = [], [], [], []
    cos_shards, sin_shards = [], []
    for bg in range(BGQ):
        bs = slice(bg * B, (bg + 1) * B)
        ckv_shards.append(np.ascontiguousarray(ckv_cache[bs, :n_cached, :]).astype(NPBF))
        ckvT_shards.append(
            np.ascontiguousarray(ckv_cache[bs, :n_cached, :].transpose(0, 2, 1)).astype(NPBF)
        )
        kpeT_shards.append(
            np.ascontiguousarray(kpe_cache[bs, :n_cached, :].transpose(0, 2, 1)).astype(NPBF)
        )
        qT_shards.append(np.ascontiguousarray(q[bs, 0, :].T).astype(NPBF))
        ckvnew_shards.append(ckv_new[bs, 0, :].astype(NPBF).reshape(1, B, C))
        ckvnewT_shards.append(np.ascontiguousarray(ckv_new[bs, 0, :].T).astype(NPBF))
        kpnew_shards.append(np.ascontiguousarray(k_pe[bs, 0, 0, :]).astype(np.float32))
        cos_shards.append(np.ascontiguousarray(cos_rep[bs]))
        sin_shards.append(np.ascontiguousarray(sin_rep[bs]))

    in_maps = []
    for core in range(N_CORES):
        hg, bg = core // BGQ, core % BGQ
        in_maps.append({
            "q_dnT": qT_shards[bg],
            "wqT": wq_shards[hg],
            "w_ukv": wukv_shards[hg],
            "w_uvT": wuv_shards[hg],
            "ckv": ckv_shards[bg],
            "ckvT": ckvT_shards[bg],
            "kpeT": kpeT_shards[bg],
            "ckv_new": ckvnew_shards[bg],
            "ckv_newT": ckvnewT_shards[bg],
            "kpe_new": kpnew_shards[bg],
            "cos_rep": cos_shards[bg],
            "sin_rep": sin_shards[bg],
        })
    return in_maps, (n_cached, B, H, bsz)


def assemble(results, meta):
    n_cached, B, H, bsz = meta
    out_full = np.empty((bsz, NUM_HEADS, V_HEAD), dtype=np.float32)
    for core in range(N_CORES):
        hg, bg = core // BGQ, core % BGQ
        out_full[bg * B:(bg + 1) * B, hg * H:(hg + 1) * H, :] = results[core]["out"]
    return out_full


def kernel(**inputs):
    in_maps, meta = prepare_in_maps(**inputs)
    n_cached, B, H, bsz = meta
    nc = _get_build(n_cached, B, H)
    res = run_bass_kernel_spmd(nc, in_maps, core_ids=list(range(N_CORES)))
    return assemble(res.results, meta)


# revision 8
# speedup vs baseline: 1.0415x; 1.0415x over previous
"""DeepseekV3 MLA decode attention kernel for 8 Trainium2 NeuronCores.

Sharding: 4 head-groups (32 heads each) x 2 batch-groups (8 batches each).
Each core computes the full attention output for its (head-group, batch-group)
tile. Weights are sharded by head, KV cache by batch. All matmul operands are
bf16 (fp32 PSUM accumulation); softmax runs in fp32.

Per-core pipeline:
  1. q = q_dn @ wq^T                (bf16 matmul, fp32 psum)
  2. RoPE on q_pe and new-token k_pe (DVE, fp32)
  3. PE transposes of q_nope/q_pe per head -> [d, b] layout
  4. absorption: q_lat^T[c,b] = w_ukv[h]^T-slices @ q_nope^T
  5. per batch: scores = q_lat.ckv^T + q_pe.kpe^T  -> exp (ACT, accum sums)
     -> transpose p -> o = p^T.T @ ckv -> scale by 1/sum
  6. out[b,d] = o^T-slices @ w_uv^T per head
"""

import sys

for _p in ("/opt/trn_rl_repo", "/root/.axon_site/_ro/trn_rl_repo"):
    if _p not in sys.path:
        sys.path.append(_p)

import numpy as np
import ml_dtypes

import concourse.bass as bass
import concourse.bacc as bacc
import concourse.tile as tile
from concourse import mybir
from concourse.bass_utils import run_bass_kernel_spmd
from concourse.masks import make_identity

BF16 = mybir.dt.bfloat16
FP8 = mybir.dt.float8e4
F32 = mybir.dt.float32
NPBF = ml_dtypes.bfloat16
NPF8 = ml_dtypes.float8_e4m3
FP8S = 16.0  # scale applied to fp8-stored tensors (q side and k side)

NUM_HEADS = 128
QK_NOPE = 128
QK_ROPE = 64
V_HEAD = 128
QD = 192  # q head dim (nope + rope)
C = 512   # kv lora rank
L = 1536  # q lora rank
SCALE = 1.0 / float(np.sqrt(192.0))

HG = 4  # head groups
BGQ = 2  # batch groups
N_CORES = 8

_BUILD_CACHE = {}


def _build(n_cached, B, H):
    """Build the per-core Bass program. Identical on all cores (pure SPMD)."""
    NT_T = n_cached // 128   # full 128-row n tiles (16)
    NCH = n_cached // 512    # 512-wide score chunks (4)
    HD = H * QD              # 6144
    LT = L // 128            # 12
    NJ = HD // 512           # 12
    assert n_cached % 512 == 0

    nc = bacc.Bacc("TRN2", target_bir_lowering=False, debug=False)

    q_dnT = nc.dram_tensor("q_dnT", [L, B], FP8, kind="ExternalInput")
    wqT = nc.dram_tensor("wqT", [L, HD], FP8, kind="ExternalInput")
    w_ukv = nc.dram_tensor("w_ukv", [H, QK_NOPE, C], BF16, kind="ExternalInput")
    w_uvT = nc.dram_tensor("w_uvT", [H, 128, 4, V_HEAD], BF16, kind="ExternalInput")
    ckv = nc.dram_tensor("ckv", [B, n_cached, C], BF16, kind="ExternalInput")
    ckvT = nc.dram_tensor("ckvT", [B, C, n_cached], FP8, kind="ExternalInput")
    kpeT = nc.dram_tensor("kpeT", [B, QK_ROPE, n_cached], FP8, kind="ExternalInput")
    ckv_new = nc.dram_tensor("ckv_new", [1, B, C], BF16, kind="ExternalInput")
    ckv_newT = nc.dram_tensor("ckv_newT", [C, B], FP8, kind="ExternalInput")
    kpe_new = nc.dram_tensor("kpe_new", [B, QK_ROPE], F32, kind="ExternalInput")
    cos_rep = nc.dram_tensor("cos_rep", [B, H * 32], F32, kind="ExternalInput")
    sin_rep = nc.dram_tensor("sin_rep", [B, H * 32], F32, kind="ExternalInput")
    out = nc.dram_tensor("out", [B, H, V_HEAD], F32, kind="ExternalOutput")

    with tile.TileContext(nc) as tc:
        # Outer (whole-kernel-lifetime) pools. The big cache-streaming pools
        # are opened first so their SBUF addresses never overlap the phase-A
        # scratch pools -> their DMAs can start at t=0.
        with (
            tc.tile_pool(name="ckvT_p", bufs=4) as ckvT_p,
            tc.tile_pool(name="ckv_p", bufs=4) as ckv_p,
            tc.tile_pool(name="kpeT_p", bufs=2) as kpeT_p,
            tc.tile_pool(name="consts", bufs=1) as consts,
            tc.tile_pool(name="persist", bufs=1) as persist,
        ):
            ident = consts.tile([128, 128], BF16)
            make_identity(nc, ident)
            cos_sb = consts.tile([B, H * 32], F32)
            nc.sync.dma_start(out=cos_sb, in_=cos_rep[:, :])
            sin_sb = consts.tile([B, H * 32], F32)
            nc.sync.dma_start(out=sin_sb, in_=sin_rep[:, :])
            kpnew_sb = consts.tile([B, QK_ROPE], F32)
            nc.sync.dma_start(out=kpnew_sb, in_=kpe_new[:, :])
            qdn_sb = consts.tile([128, LT, B], FP8)
            nc.sync.dma_start(
                out=qdn_sb, in_=q_dnT[:, :].rearrange("(t p) b -> p t b", p=128)
            )
            ckvnewT_sb = consts.tile([128, 4, B], FP8)
            nc.sync.dma_start(
                out=ckvnewT_sb, in_=ckv_newT[:, :].rearrange("(ct p) b -> p ct b", p=128)
            )
            ckvnew_sb2 = consts.tile([1, B, C], BF16)
            nc.sync.dma_start(out=ckvnew_sb2, in_=ckv_new[:, :, :])

            # persistent intermediates
            qlatT = persist.tile([128, 4, H, B], FP8)
            qpeT = persist.tile([QK_ROPE, H, B], FP8)
            knewT = persist.tile([QK_ROPE, B], FP8)

            # ---------------- Phase A: q projection, rope, transposes -------
            with (
                tc.tile_pool(name="s1a", bufs=1) as s1a,
                tc.tile_pool(name="wq_p", bufs=16) as wq_p,
                tc.tile_pool(name="wukv_p", bufs=4) as wukv_p,
                tc.tile_pool(name="ps_q", bufs=2, space="PSUM") as ps_q,
                tc.tile_pool(name="ps_t", bufs=2, space="PSUM") as ps_t,
            ):
                q_sb = s1a.tile([B, HD], BF16)
                JG = 4  # j's per wq column group
                for jg in range(NJ // JG):
                    wq_tiles = []
                    for t in range(LT):
                        wqt = wq_p.tile([128, JG * 512], FP8, tag="wq", name=f"wqt{jg}_{t}")
                        nc.sync.dma_start(
                            out=wqt,
                            in_=wqT[t * 128:(t + 1) * 128,
                                    jg * JG * 512:(jg + 1) * JG * 512],
                        )
                        wq_tiles.append(wqt)
                    for jj in range(JG):
                        j = jg * JG + jj
                        psq = ps_q.tile([B, 512], F32, tag="psq")
                        for t in range(LT):
                            nc.tensor.matmul(
                                psq, lhsT=qdn_sb[:, t, :],
                                rhs=wq_tiles[t][:, jj * 512:(jj + 1) * 512],
                                start=(t == 0), stop=(t == LT - 1),
                            )
                        nc.vector.tensor_copy(q_sb[:, j * 512:(j + 1) * 512], psq)

                qv = q_sb.rearrange("b (h d) -> b h d", d=QD)
                # rope on q_pe: interleaved pairs -> half-split rotated layout
                xpairs = qv[:, :, QK_NOPE:].rearrange("b h (i two) -> b h i two", two=2)
                xe = xpairs[:, :, :, 0]
                xo = xpairs[:, :, :, 1]
                cos3 = cos_sb.rearrange("b (h i) -> b h i", i=32)
                sin3 = sin_sb.rearrange("b (h i) -> b h i", i=32)
                qpe_bf = s1a.tile([B, H, QK_ROPE], BF16)
                tmp = s1a.tile([B, 4, H, 32], F32)
                nc.vector.tensor_mul(tmp[:, 0], xe, cos3)
                nc.vector.tensor_mul(tmp[:, 1], xo, sin3)
                nc.vector.tensor_sub(qpe_bf[:, :, 0:32], tmp[:, 0], tmp[:, 1])
                nc.vector.tensor_mul(tmp[:, 2], xo, cos3)
                nc.vector.tensor_mul(tmp[:, 3], xe, sin3)
                nc.vector.tensor_add(qpe_bf[:, :, 32:64], tmp[:, 2], tmp[:, 3])

                # rope on the new-token k_pe
                kpairs = kpnew_sb.rearrange("b (i two) -> b i two", two=2)
                kxe = kpairs[:, :, 0]
                kxo = kpairs[:, :, 1]
                kr_bf = s1a.tile([B, QK_ROPE], BF16)
                ktmp = s1a.tile([B, 4, 32], F32)
                nc.vector.tensor_mul(ktmp[:, 0], kxe, cos3[:, 0, :])
                nc.vector.tensor_mul(ktmp[:, 1], kxo, sin3[:, 0, :])
                nc.vector.tensor_sub(kr_bf[:, 0:32], ktmp[:, 0], ktmp[:, 1])
                nc.vector.tensor_mul(ktmp[:, 2], kxo, cos3[:, 0, :])
                nc.vector.tensor_mul(ktmp[:, 3], kxe, sin3[:, 0, :])
                nc.vector.tensor_add(kr_bf[:, 32:64], ktmp[:, 2], ktmp[:, 3])

                # transposes: [B, d] -> [d, B] per head
                qnT = s1a.tile([128, H, B], BF16)
                for h in range(H):
                    pt1 = ps_t.tile([128, B], BF16, tag="tr")
                    nc.tensor.transpose(pt1, qv[:, h, 0:QK_NOPE], ident[:B, :B])
                    nc.vector.tensor_copy(qnT[:, h, :], pt1)
                    pt2 = ps_t.tile([128, B], BF16, tag="tr")
                    nc.tensor.transpose(pt2[:QK_ROPE], qpe_bf[:, h, :], ident[:B, :B])
                    nc.vector.tensor_copy(qpeT[:, h, :], pt2[:QK_ROPE])
                ptk = ps_t.tile([128, B], BF16, tag="tr")
                nc.tensor.transpose(ptk[:QK_ROPE], kr_bf, ident[:B, :B])
                nc.vector.tensor_copy(knewT, ptk[:QK_ROPE])

                # absorption: q_latT[c, b] per head; weights in 8-head chunks
                HCH = 8
                for hc in range(H // HCH):
                    wut = wukv_p.tile([128, HCH, C], BF16, tag="wukv")
                    nc.sync.dma_start(
                        out=wut,
                        in_=w_ukv[hc * HCH:(hc + 1) * HCH].rearrange("h d c -> d h c"),
                    )
                    for hh in range(HCH):
                        h = hc * HCH + hh
                        for ct in range(4):
                            pa = ps_t.tile([128, B], F32, tag="abs")
                            nc.tensor.matmul(
                                pa, lhsT=wut[:, hh, ct * 128:(ct + 1) * 128],
                                rhs=qnT[:, h, :], start=True, stop=True,
                            )
                            nc.vector.tensor_copy(qlatT[:, ct, h, :], pa)

            # ---------------- Phase B: attention per batch ------------------
            with (
                tc.tile_pool(name="p_p", bufs=2) as p_p,
                tc.tile_pool(name="pT_p", bufs=2) as pT_p,
                tc.tile_pool(name="o_p", bufs=2) as o_p,
                tc.tile_pool(name="oT_p", bufs=1) as oT_p,
                tc.tile_pool(name="sum_p", bufs=2) as sum_p,
                tc.tile_pool(name="wuv_p", bufs=4) as wuv_p,
                tc.tile_pool(name="outs_p", bufs=1) as outs_p,
                tc.tile_pool(name="ps_s", bufs=2, space="PSUM") as ps_s,
                tc.tile_pool(name="ps_pt", bufs=2, space="PSUM") as ps_pt,
                tc.tile_pool(name="ps_o", bufs=1, space="PSUM") as ps_o,
                tc.tile_pool(name="ps_r", bufs=2, space="PSUM") as ps_r,
            ):
                oT = oT_p.tile([128, 4, H, B], BF16)
                for b in range(B):
                    ckvT_t = ckvT_p.tile([128, 4, n_cached], FP8, tag="ckvT")
                    nc.sync.dma_start(
                        out=ckvT_t,
                        in_=ckvT[b].rearrange("(ct p) n -> p ct n", p=128),
                    )
                    kpeT_t = kpeT_p.tile([QK_ROPE, n_cached], FP8, tag="kpeT")
                    nc.sync.dma_start(out=kpeT_t, in_=kpeT[b])

                    p_bf = p_p.tile([32, n_cached], BF16, tag="p")
                    p_tail = p_p.tile([32, 1], BF16, tag="ptail")
                    sums = sum_p.tile([32, 8], F32, tag="sums")
                    # scores + exp, 512-wide chunks
                    for nch in range(NCH):
                        pss = ps_s.tile([32, 512], F32, tag="s")
                        for ct in range(4):
                            nc.tensor.matmul(
                                pss,
                                lhsT=qlatT[:, ct, :, b],
                                rhs=ckvT_t[:, ct, nch * 512:(nch + 1) * 512],
                                start=(ct == 0), stop=False,
                            )
                        nc.tensor.matmul(
                            pss, lhsT=qpeT[:, :, b],
                            rhs=kpeT_t[:, nch * 512:(nch + 1) * 512],
                            start=False, stop=True,
                        )
                        nc.scalar.activation(
                            p_bf[:, nch * 512:(nch + 1) * 512], pss,
                            mybir.ActivationFunctionType.Exp,
                            scale=SCALE / (FP8S * FP8S * FP8S), accum_out=sums[:, nch:nch + 1],
                        )
                    # new-token column
                    pst = ps_s.tile([32, 512], F32, tag="s")
                    for ct in range(4):
                        nc.tensor.matmul(
                            pst[:, 0:1], lhsT=qlatT[:, ct, :, b],
                            rhs=ckvnewT_sb[:, ct, b:b + 1],
                            start=(ct == 0), stop=False,
                        )
                    nc.tensor.matmul(
                        pst[:, 0:1], lhsT=qpeT[:, :, b], rhs=knewT[:, b:b + 1],
                        start=False, stop=True,
                    )
                    nc.scalar.activation(
                        p_tail, pst[:, 0:1],
                        mybir.ActivationFunctionType.Exp,
                        scale=SCALE / (FP8S * FP8S * FP8S), accum_out=sums[:, NCH:NCH + 1],
                    )
                    # 1 / sum
                    ssum = sum_p.tile([32, 1], F32, tag="ssum")
                    nc.vector.reduce_sum(ssum, sums[:, 0:NCH + 1], axis=mybir.AxisListType.X)
                    rcp = sum_p.tile([32, 1], F32, tag="rcp")
                    nc.vector.reciprocal(rcp, ssum)

                    # transpose p -> pT tiles
                    pT = pT_p.tile([128, NT_T, 32], BF16, tag="pT")
                    for nt in range(NT_T):
                        ptp = ps_pt.tile([128, 32], BF16, tag="pt")
                        nc.tensor.transpose(
                            ptp, p_bf[:, nt * 128:(nt + 1) * 128], ident[:32, :32]
                        )
                        nc.vector.tensor_copy(pT[:, nt, :], ptp)
                    ptt = ps_pt.tile([128, 32], BF16, tag="pt")
                    nc.tensor.transpose(ptt[0:1], p_tail, ident[:32, :32])
                    pT_tail = pT_p.tile([1, 32], BF16, tag="pTt")
                    nc.vector.tensor_copy(pT_tail, ptt[0:1])

                    # o = p @ ckv   (accumulate over n tiles)
                    pso = ps_o.tile([32, C], F32, tag="o")
                    for g in range(NT_T // 4):
                        ckv_t = ckv_p.tile([128, 4, C], BF16, tag="ckv")
                        nc.sync.dma_start(
                            out=ckv_t,
                            in_=ckv[b, g * 512:(g + 1) * 512, :].rearrange(
                                "(nt p) c -> p nt c", p=128
                            ),
                        )
                        for k in range(4):
                            nt = g * 4 + k
                            nc.tensor.matmul(
                                pso, lhsT=pT[:, nt, :], rhs=ckv_t[:, k, :],
                                start=(nt == 0), stop=False,
                            )
                    nc.tensor.matmul(
                        pso, lhsT=pT_tail, rhs=ckvnew_sb2[:, b, :],
                        start=False, stop=True,
                    )
                    # o / sum -> bf16
                    o_bf = o_p.tile([32, C], BF16, tag="obf")
                    nc.vector.tensor_scalar_mul(o_bf, pso, rcp)
                    # transpose o -> oT[:, ct, :, b]
                    for ct in range(4):
                        pto = ps_pt.tile([128, 32], BF16, tag="pt")
                        nc.tensor.transpose(
                            pto, o_bf[:, ct * 128:(ct + 1) * 128], ident[:32, :32]
                        )
                        nc.vector.tensor_copy(oT[:, ct, :, b], pto)

                # ---------------- output projection ------------------------
                out_sb = outs_p.tile([B, H, V_HEAD], F32)
                HCH = 8
                for hc in range(H // HCH):
                    wvt = wuv_p.tile([128, HCH, 4, V_HEAD], BF16, tag="wuv")
                    nc.sync.dma_start(
                        out=wvt,
                        in_=w_uvT[hc * HCH:(hc + 1) * HCH].rearrange("h p ct d -> p h ct d"),
                    )
                    for hh in range(HCH):
                        h = hc * HCH + hh
                        psr = ps_r.tile([B, V_HEAD], F32, tag="r")
                        for ct in range(4):
                            nc.tensor.matmul(
                                psr, lhsT=oT[:, ct, h, :], rhs=wvt[:, hh, ct, :],
                                start=(ct == 0), stop=(ct == 3),
                            )
                        nc.vector.tensor_copy(out_sb[:, h, :], psr)
                nc.sync.dma_start(out=out[:, :, :], in_=out_sb)

    nc.compile()
    return nc


def _get_build(n_cached, B, H):
    key = (n_cached, B, H)
    if key not in _BUILD_CACHE:
        _BUILD_CACHE[key] = _build(n_cached, B, H)
    return _BUILD_CACHE[key]


def prepare_in_maps(**inputs):
    """Host-side sharding / layout prep. Returns (in_maps, meta)."""
    q = np.asarray(inputs["q_normed_dn"], dtype=np.float32)      # [16,1,1536]
    ckv_new = np.asarray(inputs["compressed_kv"], dtype=np.float32)  # [16,1,512]
    k_pe = np.asarray(inputs["k_pe"], dtype=np.float32)          # [16,1,1,64]
    pos = np.asarray(inputs["position_ids"]).astype(np.int64)    # [16,1]
    start_pos = int(inputs["start_pos"])
    ckv_cache = np.asarray(inputs["ckv_cache"], dtype=np.float32)
    kpe_cache = np.asarray(inputs["k_pe_cache"], dtype=np.float32)
    sin_c = np.asarray(inputs["sin_cache"], dtype=np.float32)
    cos_c = np.asarray(inputs["cos_cache"], dtype=np.float32)
    wkv_b = np.asarray(inputs["wkv_b"], dtype=np.float32)        # [128,256,512]
    wq_b = np.asarray(inputs["wq_b"], dtype=np.float32)          # [24576,1536]

    bsz = q.shape[0]
    B = bsz // BGQ
    H = NUM_HEADS // HG
    n_cached = start_pos

    cos_g = cos_c[pos[:, 0]][:, :32]                             # [16,32]
    sin_g = sin_c[pos[:, 0]][:, :32]
    cos_rep = np.tile(cos_g, (1, H)).astype(np.float32)          # [16,H*32]
    sin_rep = np.tile(sin_g, (1, H)).astype(np.float32)

    wq_r = wq_b.reshape(NUM_HEADS, QD, L)

    # per head-group weights
    wq_shards, wukv_shards, wuv_shards = [], [], []
    for hg in range(HG):
        hs = slice(hg * H, (hg + 1) * H)
        wq_shards.append(
            np.ascontiguousarray(wq_r[hs].reshape(H * QD, L).T * FP8S).astype(NPF8)
        )
        wukv_shards.append(np.ascontiguousarray(wkv_b[hs, :QK_NOPE, :]).astype(NPBF))
        # w_uv pre-arranged to the SBUF tile layout [H, p, ct, d] so every
        # DMA descriptor run is >= 1KB contiguous
        wuvT = wkv_b[hs, QK_NOPE:, :].transpose(0, 2, 1)          # [H, C, D]
        wuv_shards.append(
            np.ascontiguousarray(
                wuvT.reshape(H, 4, 128, V_HEAD).transpose(0, 2, 1, 3)
            ).astype(NPBF)
        )

    # per batch-group caches
    ckv_shards, ckvT_shards, kpeT_shards = [], [], []
    qT_shards, ckvnew_shards, ckvnewT_shards, kpnew_shards = [], [], [], []
    cos_shards, sin_shards = [], []
    for bg in range(BGQ):
        bs = slice(bg * B, (bg + 1) * B)
        ckv_shards.append(np.ascontiguousarray(ckv_cache[bs, :n_cached, :]).astype(NPBF))
        ckvT_shards.append(
            np.ascontiguousarray(ckv_cache[bs, :n_cached, :].transpose(0, 2, 1) * FP8S).astype(NPF8)
        )
        kpeT_shards.append(
            np.ascontiguousarray(kpe_cache[bs, :n_cached, :].transpose(0, 2, 1) * FP8S).astype(NPF8)
        )
        qT_shards.append(np.ascontiguousarray(q[bs, 0, :].T * FP8S).astype(NPF8))
        ckvnew_shards.append(ckv_new[bs, 0, :].astype(NPBF).reshape(1, B, C))
        ckvnewT_shards.append(np.ascontiguousarray(ckv_new[bs, 0, :].T * FP8S).astype(NPF8))
        kpnew_shards.append(np.ascontiguousarray(k_pe[bs, 0, 0, :] * FP8S).astype(np.float32))
        cos_shards.append(np.ascontiguousarray(cos_rep[bs]))
        sin_shards.append(np.ascontiguousarray(sin_rep[bs]))

    in_maps = []
    for core in range(N_CORES):
        hg, bg = core // BGQ, core % BGQ
        in_maps.append({
            "q_dnT": qT_shards[bg],
            "wqT": wq_shards[hg],
            "w_ukv": wukv_shards[hg],
            "w_uvT": wuv_shards[hg],
            "ckv": ckv_shards[bg],
            "ckvT": ckvT_shards[bg],
            "kpeT": kpeT_shards[bg],
            "ckv_new": ckvnew_shards[bg],
            "ckv_newT": ckvnewT_shards[bg],
            "kpe_new": kpnew_shards[bg],
            "cos_rep": cos_shards[bg],
            "sin_rep": sin_shards[bg],
        })
    return in_maps, (n_cached, B, H, bsz)


def assemble(results, meta):
    n_cached, B, H, bsz = meta
    out_full = np.empty((bsz, NUM_HEADS, V_HEAD), dtype=np.float32)
    for core in range(N_CORES):
        hg, bg = core // BGQ, core % BGQ
        out_full[bg * B:(bg + 1) * B, hg * H:(hg + 1) * H, :] = results[core]["out"]
    return out_full


def kernel(**inputs):
    in_maps, meta = prepare_in_maps(**inputs)
    n_cached, B, H, bsz = meta
    nc = _get_build(n_cached, B, H)
    res = run_bass_kernel_spmd(nc, in_maps, core_ids=list(range(N_CORES)))
    return assemble(res.results, meta)


# revision 12
# speedup vs baseline: 8.3301x; 7.9982x over previous
"""DeepseekV3 MLA decode attention kernel for 8 Trainium2 NeuronCores.

Sharding: 4 head-groups (32 heads each) x 2 batch-groups (8 batches each).
Each core computes the full attention output for its (head-group, batch-group)
tile. Weights are sharded by head, KV cache by batch. All matmul operands are
bf16 (fp32 PSUM accumulation); softmax runs in fp32.

Per-core pipeline:
  1. q = q_dn @ wq^T                (bf16 matmul, fp32 psum)
  2. RoPE on q_pe and new-token k_pe (DVE, fp32)
  3. PE transposes of q_nope/q_pe per head -> [d, b] layout
  4. absorption: q_lat^T[c,b] = w_ukv[h]^T-slices @ q_nope^T
  5. per batch: scores = q_lat.ckv^T + q_pe.kpe^T  -> exp (ACT, accum sums)
     -> transpose p -> o = p^T.T @ ckv -> scale by 1/sum
  6. out[b,d] = o^T-slices @ w_uv^T per head
"""

import sys

for _p in ("/opt/trn_rl_repo", "/root/.axon_site/_ro/trn_rl_repo"):
    if _p not in sys.path:
        sys.path.append(_p)

import numpy as np
import ml_dtypes

import concourse.bass as bass
import concourse.bacc as bacc
import concourse.tile as tile
from concourse import mybir
from concourse.bass_utils import run_bass_kernel_spmd
from concourse.masks import make_identity

BF16 = mybir.dt.bfloat16
FP8 = mybir.dt.float8e4
F32 = mybir.dt.float32
NPBF = ml_dtypes.bfloat16
NPF8 = ml_dtypes.float8_e4m3
FP8S = 16.0  # scale applied to fp8-stored tensors (q side and k side)

NUM_HEADS = 128
QK_NOPE = 128
QK_ROPE = 64
V_HEAD = 128
QD = 192  # q head dim (nope + rope)
C = 512   # kv lora rank
L = 1536  # q lora rank
SCALE = 1.0 / float(np.sqrt(192.0))

HG = 4  # head groups
BGQ = 2  # batch groups
N_CORES = 8

_BUILD_CACHE = {}


def _build(n_cached, B, H):
    """Build the per-core Bass program. Identical on all cores (pure SPMD)."""
    NT_T = n_cached // 128   # full 128-row n tiles (16)
    NCH = n_cached // 512    # 512-wide score chunks (4)
    HD = H * QD              # 6144
    LT = L // 128            # 12
    NJ = HD // 512           # 12
    assert n_cached % 512 == 0

    nc = bacc.Bacc("TRN2", target_bir_lowering=False, debug=False)

    q_dnT = nc.dram_tensor("q_dnT", [L, B], FP8, kind="ExternalInput")
    wqT = nc.dram_tensor("wqT", [L, HD], FP8, kind="ExternalInput")
    w_ukv = nc.dram_tensor("w_ukv", [H, QK_NOPE, C], FP8, kind="ExternalInput")
    w_uvT = nc.dram_tensor("w_uvT", [H, 128, 4, V_HEAD], BF16, kind="ExternalInput")
    ckv = nc.dram_tensor("ckv", [B, n_cached, C], BF16, kind="ExternalInput")
    ckvT = nc.dram_tensor("ckvT", [B, C, n_cached], FP8, kind="ExternalInput")
    kpeT = nc.dram_tensor("kpeT", [B, QK_ROPE, n_cached], FP8, kind="ExternalInput")
    ckv_new = nc.dram_tensor("ckv_new", [1, B, C], BF16, kind="ExternalInput")
    ckv_newT = nc.dram_tensor("ckv_newT", [C, B], FP8, kind="ExternalInput")
    kpe_new = nc.dram_tensor("kpe_new", [B, QK_ROPE], F32, kind="ExternalInput")
    cos_rep = nc.dram_tensor("cos_rep", [B, H * 32], F32, kind="ExternalInput")
    sin_rep = nc.dram_tensor("sin_rep", [B, H * 32], F32, kind="ExternalInput")
    out = nc.dram_tensor("out", [B, H, V_HEAD], F32, kind="ExternalOutput")

    with tile.TileContext(nc) as tc:
        # Outer (whole-kernel-lifetime) pools. The big cache-streaming pools
        # are opened first so their SBUF addresses never overlap the phase-A
        # scratch pools -> their DMAs can start at t=0.
        with (
            tc.tile_pool(name="ckvT_p", bufs=4) as ckvT_p,
            tc.tile_pool(name="ckv_p", bufs=4) as ckv_p,
            tc.tile_pool(name="kpeT_p", bufs=2) as kpeT_p,
            tc.tile_pool(name="consts", bufs=1) as consts,
            tc.tile_pool(name="persist", bufs=1) as persist,
        ):
            ident = consts.tile([128, 128], BF16)
            make_identity(nc, ident)
            cos_sb = consts.tile([B, H * 32], F32)
            nc.sync.dma_start(out=cos_sb, in_=cos_rep[:, :])
            sin_sb = consts.tile([B, H * 32], F32)
            nc.sync.dma_start(out=sin_sb, in_=sin_rep[:, :])
            kpnew_sb = consts.tile([B, QK_ROPE], F32)
            nc.sync.dma_start(out=kpnew_sb, in_=kpe_new[:, :])
            qdn_sb = consts.tile([128, LT, B], FP8)
            nc.sync.dma_start(
                out=qdn_sb, in_=q_dnT[:, :].rearrange("(t p) b -> p t b", p=128)
            )
            ckvnewT_sb = consts.tile([128, 4, B], FP8)
            nc.sync.dma_start(
                out=ckvnewT_sb, in_=ckv_newT[:, :].rearrange("(ct p) b -> p ct b", p=128)
            )
            ckvnew_sb2 = consts.tile([1, B, C], BF16)
            nc.sync.dma_start(out=ckvnew_sb2, in_=ckv_new[:, :, :])

            # persistent intermediates
            qlatT = persist.tile([128, H, 4, B], FP8)
            qpeT = persist.tile([QK_ROPE, H, B], FP8)
            knewT = persist.tile([QK_ROPE, B], FP8)

            # ---------------- Phase A: q projection, rope, transposes -------
            with (
                tc.tile_pool(name="s1a", bufs=1) as s1a,
                tc.tile_pool(name="wq_p", bufs=16) as wq_p,
                tc.tile_pool(name="wukv_p", bufs=4) as wukv_p,
                tc.tile_pool(name="ps_q", bufs=2, space="PSUM") as ps_q,
                tc.tile_pool(name="ps_t", bufs=2, space="PSUM") as ps_t,
            ):
                q_sb = s1a.tile([B, HD], BF16)
                JG = 4  # j's per wq column group
                for jg in range(NJ // JG):
                    wq_tiles = []
                    for t in range(LT):
                        wqt = wq_p.tile([128, JG * 512], FP8, tag="wq", name=f"wqt{jg}_{t}")
                        nc.sync.dma_start(
                            out=wqt,
                            in_=wqT[t * 128:(t + 1) * 128,
                                    jg * JG * 512:(jg + 1) * JG * 512],
                        )
                        wq_tiles.append(wqt)
                    for jj in range(JG):
                        j = jg * JG + jj
                        psq = ps_q.tile([B, 512], F32, tag="psq")
                        for t in range(LT):
                            nc.tensor.matmul(
                                psq, lhsT=qdn_sb[:, t, :],
                                rhs=wq_tiles[t][:, jj * 512:(jj + 1) * 512],
                                start=(t == 0), stop=(t == LT - 1),
                            )
                        nc.vector.tensor_copy(q_sb[:, j * 512:(j + 1) * 512], psq)

                qv = q_sb.rearrange("b (h d) -> b h d", d=QD)
                # rope on q_pe: interleaved pairs -> half-split rotated layout
                xpairs = qv[:, :, QK_NOPE:].rearrange("b h (i two) -> b h i two", two=2)
                xe = xpairs[:, :, :, 0]
                xo = xpairs[:, :, :, 1]
                cos3 = cos_sb.rearrange("b (h i) -> b h i", i=32)
                sin3 = sin_sb.rearrange("b (h i) -> b h i", i=32)
                qpe_bf = s1a.tile([B, H, QK_ROPE], BF16)
                tmp = s1a.tile([B, 4, H, 32], F32)
                nc.vector.tensor_mul(tmp[:, 0], xe, cos3)
                nc.vector.tensor_mul(tmp[:, 1], xo, sin3)
                nc.vector.tensor_sub(qpe_bf[:, :, 0:32], tmp[:, 0], tmp[:, 1])
                nc.vector.tensor_mul(tmp[:, 2], xo, cos3)
                nc.vector.tensor_mul(tmp[:, 3], xe, sin3)
                nc.vector.tensor_add(qpe_bf[:, :, 32:64], tmp[:, 2], tmp[:, 3])

                # rope on the new-token k_pe
                kpairs = kpnew_sb.rearrange("b (i two) -> b i two", two=2)
                kxe = kpairs[:, :, 0]
                kxo = kpairs[:, :, 1]
                kr_bf = s1a.tile([B, QK_ROPE], BF16)
                ktmp = s1a.tile([B, 4, 32], F32)
                nc.vector.tensor_mul(ktmp[:, 0], kxe, cos3[:, 0, :])
                nc.vector.tensor_mul(ktmp[:, 1], kxo, sin3[:, 0, :])
                nc.vector.tensor_sub(kr_bf[:, 0:32], ktmp[:, 0], ktmp[:, 1])
                nc.vector.tensor_mul(ktmp[:, 2], kxo, cos3[:, 0, :])
                nc.vector.tensor_mul(ktmp[:, 3], kxe, sin3[:, 0, :])
                nc.vector.tensor_add(kr_bf[:, 32:64], ktmp[:, 2], ktmp[:, 3])

                # transposes: [B, d] -> [d, B], grouped 8 heads per psum tile
                TCH = 8
                qnT = s1a.tile([128, H, B], FP8)
                for hc in range(H // TCH):
                    ptn = ps_t.tile([128, TCH, B], BF16, tag="tr")
                    ptp2 = ps_t.tile([128, TCH, B], BF16, tag="tr")
                    for hh in range(TCH):
                        h = hc * TCH + hh
                        nc.tensor.transpose(ptn[:, hh, :], qv[:, h, 0:QK_NOPE], ident[:B, :B])
                        nc.tensor.transpose(ptp2[:QK_ROPE, hh, :], qpe_bf[:, h, :], ident[:B, :B])
                    nc.vector.tensor_copy(qnT[:, hc * TCH:(hc + 1) * TCH, :], ptn)
                    nc.vector.tensor_copy(qpeT[:, hc * TCH:(hc + 1) * TCH, :], ptp2[:QK_ROPE])
                ptk = ps_t.tile([128, TCH, B], BF16, tag="tr")
                nc.tensor.transpose(ptk[:QK_ROPE, 0, :], kr_bf, ident[:B, :B])
                nc.vector.tensor_copy(knewT, ptk[:QK_ROPE, 0, :])

                # absorption: q_latT[c, b]; 8 heads x 4 ct per psum tile
                HCH = 8
                for hc in range(H // HCH):
                    wut = wukv_p.tile([128, HCH, C], FP8, tag="wukv")
                    nc.sync.dma_start(
                        out=wut,
                        in_=w_ukv[hc * HCH:(hc + 1) * HCH].rearrange("h d c -> d h c"),
                    )
                    pa = ps_t.tile([128, HCH, 4, B], F32, tag="abs")
                    for hh in range(HCH):
                        h = hc * HCH + hh
                        for ct in range(4):
                            nc.tensor.matmul(
                                pa[:, hh, ct, :],
                                lhsT=wut[:, hh, ct * 128:(ct + 1) * 128],
                                rhs=qnT[:, h, :], start=True, stop=True,
                            )
                    nc.vector.tensor_scalar_mul(
                        qlatT[:, hc * HCH:(hc + 1) * HCH, :, :], pa, 1.0 / FP8S
                    )

            # ---------------- Phase B: attention per batch ------------------
            with (
                tc.tile_pool(name="p_p", bufs=2) as p_p,
                tc.tile_pool(name="pT_p", bufs=2) as pT_p,
                tc.tile_pool(name="o_p", bufs=2) as o_p,
                tc.tile_pool(name="oT_p", bufs=1) as oT_p,
                tc.tile_pool(name="sum_p", bufs=2) as sum_p,
                tc.tile_pool(name="wuv_p", bufs=4) as wuv_p,
                tc.tile_pool(name="outs_p", bufs=1) as outs_p,
                tc.tile_pool(name="ps_s", bufs=2, space="PSUM") as ps_s,
                tc.tile_pool(name="ps_pt", bufs=2, space="PSUM") as ps_pt,
                tc.tile_pool(name="ps_o", bufs=1, space="PSUM") as ps_o,
                tc.tile_pool(name="ps_r", bufs=2, space="PSUM") as ps_r,
            ):
                oT = oT_p.tile([128, 4, H, B], BF16)
                kpe_tiles = {}
                for b in range(B):
                    ckvT_t = ckvT_p.tile([128, 4, n_cached], FP8, tag="ckvT")
                    nc.sync.dma_start(
                        out=ckvT_t,
                        in_=ckvT[b].rearrange("(ct p) n -> p ct n", p=128),
                    )
                    if b % 2 == 0:
                        kpeT_t2 = kpeT_p.tile([QK_ROPE, 2, n_cached], FP8, tag="kpeT")
                        nc.sync.dma_start(
                            out=kpeT_t2,
                            in_=kpeT[b:b + 2].rearrange("b j n -> j b n"),
                        )
                    kpeT_t = kpeT_t2[:, b % 2, :]

                    p_bf = p_p.tile([32, n_cached], BF16, tag="p")
                    p_tail = p_p.tile([32, 1], BF16, tag="ptail")
                    sums = sum_p.tile([32, 8], F32, tag="sums")
                    # scores + exp, 512-wide chunks
                    for nch in range(NCH):
                        pss = ps_s.tile([32, 512], F32, tag="s")
                        for ct in range(4):
                            nc.tensor.matmul(
                                pss,
                                lhsT=qlatT[:, :, ct, b],
                                rhs=ckvT_t[:, ct, nch * 512:(nch + 1) * 512],
                                start=(ct == 0), stop=False,
                            )
                        nc.tensor.matmul(
                            pss, lhsT=qpeT[:, :, b],
                            rhs=kpeT_t[:, nch * 512:(nch + 1) * 512],
                            start=False, stop=True,
                        )
                        nc.scalar.activation(
                            p_bf[:, nch * 512:(nch + 1) * 512], pss,
                            mybir.ActivationFunctionType.Exp,
                            scale=SCALE / (FP8S * FP8S * FP8S), accum_out=sums[:, nch:nch + 1],
                        )
                    # new-token column
                    pst = ps_s.tile([32, 512], F32, tag="s")
                    for ct in range(4):
                        nc.tensor.matmul(
                            pst[:, 0:1], lhsT=qlatT[:, :, ct, b],
                            rhs=ckvnewT_sb[:, ct, b:b + 1],
                            start=(ct == 0), stop=False,
                        )
                    nc.tensor.matmul(
                        pst[:, 0:1], lhsT=qpeT[:, :, b], rhs=knewT[:, b:b + 1],
                        start=False, stop=True,
                    )
                    nc.scalar.activation(
                        p_tail, pst[:, 0:1],
                        mybir.ActivationFunctionType.Exp,
                        scale=SCALE / (FP8S * FP8S * FP8S), accum_out=sums[:, NCH:NCH + 1],
                    )
                    # 1 / sum
                    ssum = sum_p.tile([32, 1], F32, tag="ssum")
                    nc.vector.reduce_sum(ssum, sums[:, 0:NCH + 1], axis=mybir.AxisListType.X)
                    rcp = sum_p.tile([32, 1], F32, tag="rcp")
                    nc.vector.reciprocal(rcp, ssum)

                    # transpose p -> pT tiles (4 per psum tile)
                    pT = pT_p.tile([128, NT_T, 32], BF16, tag="pT")
                    for g in range(NT_T // 4):
                        ptp = ps_pt.tile([128, 4, 32], BF16, tag="pt")
                        for k in range(4):
                            nt = g * 4 + k
                            nc.tensor.transpose(
                                ptp[:, k, :], p_bf[:, nt * 128:(nt + 1) * 128],
                                ident[:32, :32],
                            )
                        nc.vector.tensor_copy(pT[:, g * 4:(g + 1) * 4, :], ptp)
                    ptt = ps_pt.tile([128, 4, 32], BF16, tag="pt")
                    nc.tensor.transpose(ptt[0:1, 0, :], p_tail, ident[:32, :32])
                    pT_tail = pT_p.tile([1, 32], BF16, tag="pTt")
                    nc.vector.tensor_copy(pT_tail, ptt[0:1, 0, :])

                    # o = p @ ckv   (accumulate over n tiles)
                    pso = ps_o.tile([32, C], F32, tag="o")
                    for g in range(NT_T // 4):
                        ckv_t = ckv_p.tile([128, 4, C], BF16, tag="ckv")
                        nc.sync.dma_start(
                            out=ckv_t,
                            in_=ckv[b, g * 512:(g + 1) * 512, :].rearrange(
                                "(nt p) c -> p nt c", p=128
                            ),
                        )
                        for k in range(4):
                            nt = g * 4 + k
                            nc.tensor.matmul(
                                pso, lhsT=pT[:, nt, :], rhs=ckv_t[:, k, :],
                                start=(nt == 0), stop=False,
                            )
                    nc.tensor.matmul(
                        pso, lhsT=pT_tail, rhs=ckvnew_sb2[:, b, :],
                        start=False, stop=True,
                    )
                    # o / sum -> bf16
                    o_bf = o_p.tile([32, C], BF16, tag="obf")
                    nc.vector.tensor_scalar_mul(o_bf, pso, rcp)
                    # transpose o -> oT[:, :, :, b]
                    pto = ps_pt.tile([128, 4, 32], BF16, tag="pt")
                    for ct in range(4):
                        nc.tensor.transpose(
                            pto[:, ct, :], o_bf[:, ct * 128:(ct + 1) * 128],
                            ident[:32, :32],
                        )
                    nc.vector.tensor_copy(oT[:, :, :, b], pto)

                # ---------------- output projection ------------------------
                out_sb = outs_p.tile([B, H, V_HEAD], F32)
                HCH = 8
                for hc in range(H // HCH):
                    wvt = wuv_p.tile([128, HCH, 4, V_HEAD], BF16, tag="wuv")
                    nc.sync.dma_start(
                        out=wvt,
                        in_=w_uvT[hc * HCH:(hc + 1) * HCH].rearrange("h p ct d -> p h ct d"),
                    )
                    for hh in range(HCH):
                        h = hc * HCH + hh
                        psr = ps_r.tile([B, V_HEAD], F32, tag="r")
                        for ct in range(4):
                            nc.tensor.matmul(
                                psr, lhsT=oT[:, ct, h, :], rhs=wvt[:, hh, ct, :],
                                start=(ct == 0), stop=(ct == 3),
                            )
                        nc.vector.tensor_copy(out_sb[:, h, :], psr)
                    nc.sync.dma_start(
                        out=out[:, hc * HCH:(hc + 1) * HCH, :],
                        in_=out_sb[:, hc * HCH:(hc + 1) * HCH, :],
                    )

    nc.compile()
    return nc


def _get_build(n_cached, B, H):
    key = (n_cached, B, H)
    if key not in _BUILD_CACHE:
        _BUILD_CACHE[key] = _build(n_cached, B, H)
    return _BUILD_CACHE[key]


def prepare_in_maps(**inputs):
    """Host-side sharding / layout prep. Returns (in_maps, meta)."""
    q = np.asarray(inputs["q_normed_dn"], dtype=np.float32)      # [16,1,1536]
    ckv_new = np.asarray(inputs["compressed_kv"], dtype=np.float32)  # [16,1,512]
    k_pe = np.asarray(inputs["k_pe"], dtype=np.float32)          # [16,1,1,64]
    pos = np.asarray(inputs["position_ids"]).astype(np.int64)    # [16,1]
    start_pos = int(inputs["start_pos"])
    ckv_cache = np.asarray(inputs["ckv_cache"], dtype=np.float32)
    kpe_cache = np.asarray(inputs["k_pe_cache"], dtype=np.float32)
    sin_c = np.asarray(inputs["sin_cache"], dtype=np.float32)
    cos_c = np.asarray(inputs["cos_cache"], dtype=np.float32)
    wkv_b = np.asarray(inputs["wkv_b"], dtype=np.float32)        # [128,256,512]
    wq_b = np.asarray(inputs["wq_b"], dtype=np.float32)          # [24576,1536]

    bsz = q.shape[0]
    B = bsz // BGQ
    H = NUM_HEADS // HG
    n_cached = start_pos

    cos_g = cos_c[pos[:, 0]][:, :32]                             # [16,32]
    sin_g = sin_c[pos[:, 0]][:, :32]
    cos_rep = np.tile(cos_g, (1, H)).astype(np.float32)          # [16,H*32]
    sin_rep = np.tile(sin_g, (1, H)).astype(np.float32)

    wq_r = wq_b.reshape(NUM_HEADS, QD, L)

    # per head-group weights
    wq_shards, wukv_shards, wuv_shards = [], [], []
    for hg in range(HG):
        hs = slice(hg * H, (hg + 1) * H)
        wq_shards.append(
            np.ascontiguousarray(wq_r[hs].reshape(H * QD, L).T * FP8S).astype(NPF8)
        )
        wukv_shards.append(np.ascontiguousarray(wkv_b[hs, :QK_NOPE, :] * FP8S).astype(NPF8))
        # w_uv pre-arranged to the SBUF tile layout [H, p, ct, d] so every
        # DMA descriptor run is >= 1KB contiguous
        wuvT = wkv_b[hs, QK_NOPE:, :].transpose(0, 2, 1)          # [H, C, D]
        wuv_shards.append(
            np.ascontiguousarray(
                wuvT.reshape(H, 4, 128, V_HEAD).transpose(0, 2, 1, 3)
            ).astype(NPBF)
        )

    # per batch-group caches
    ckv_shards, ckvT_shards, kpeT_shards = [], [], []
    qT_shards, ckvnew_shards, ckvnewT_shards, kpnew_shards = [], [], [], []
    cos_shards, sin_shards = [], []
    for bg in range(BGQ):
        bs = slice(bg * B, (bg + 1) * B)
        ckv_shards.append(np.ascontiguousarray(ckv_cache[bs, :n_cached, :]).astype(NPBF))
        ckvT_shards.append(
            np.ascontiguousarray(ckv_cache[bs, :n_cached, :].transpose(0, 2, 1) * FP8S).astype(NPF8)
        )
        kpeT_shards.append(
            np.ascontiguousarray(kpe_cache[bs, :n_cached, :].transpose(0, 2, 1) * FP8S).astype(NPF8)
        )
        qT_shards.append(np.ascontiguousarray(q[bs, 0, :].T * FP8S).astype(NPF8))
        ckvnew_shards.append(ckv_new[bs, 0, :].astype(NPBF).reshape(1, B, C))
        ckvnewT_shards.append(np.ascontiguousarray(ckv_new[bs, 0, :].T * FP8S).astype(NPF8))
        kpnew_shards.append(np.ascontiguousarray(k_pe[bs, 0, 0, :] * FP8S).astype(np.float32))
        cos_shards.append(np.ascontiguousarray(cos_rep[bs]))
        sin_shards.append(np.ascontiguousarray(sin_rep[bs]))

    in_maps = []
    for core in range(N_CORES):
        hg, bg = core // BGQ, core % BGQ
        in_maps.append({
            "q_dnT": qT_shards[bg],
            "wqT": wq_shards[hg],
            "w_ukv": wukv_shards[hg],
            "w_uvT": wuv_shards[hg],
            "ckv": ckv_shards[bg],
            "ckvT": ckvT_shards[bg],
            "kpeT": kpeT_shards[bg],
            "ckv_new": ckvnew_shards[bg],
            "ckv_newT": ckvnewT_shards[bg],
            "kpe_new": kpnew_shards[bg],
            "cos_rep": cos_shards[bg],
            "sin_rep": sin_shards[bg],
        })
    return in_maps, (n_cached, B, H, bsz)


def assemble(results, meta):
    n_cached, B, H, bsz = meta
    out_full = np.empty((bsz, NUM_HEADS, V_HEAD), dtype=np.float32)
    for core in range(N_CORES):
        hg, bg = core // BGQ, core % BGQ
        out_full[bg * B:(bg + 1) * B, hg * H:(hg + 1) * H, :] = results[core]["out"]
    return out_full


def kernel(**inputs):
    in_maps, meta = prepare_in_maps(**inputs)
    n_cached, B, H, bsz = meta
    nc = _get_build(n_cached, B, H)
    res = run_bass_kernel_spmd(nc, in_maps, core_ids=list(range(N_CORES)))
    return assemble(res.results, meta)
